# revision 48
# baseline (speedup 1.0000x reference)
"""Bass/Trainium2 kernel for nn_Context_RGR_20718922235945 (retrieval_knn).

No-collective design (8 NeuronCores, gallery sharded along N):
  host : normalize t and gallery, cast to bf16, transpose for DMA-friendly
         layouts, shard gallery N across 8 cores.
  core : sims slab  [128, 8192] = t_n @ g_n_shard.T   (PE, bf16 -> f32 psum)
         local top-8 values+indices per batch row      (DVE max / max_index)
         gather all 1024 candidate gallery rows        (gpsimd dma_gather)
         prod rows = g_row * s_row (elementwise)       (DVE)
         DMA out: top-8 values, top-8 local indices, 1024 prod rows (bf16)
  host : merge the 8 local top-8 lists -> global top-5 per row (the
         gather/unshard "reduce" step), bottom-256 membership per winner row,
         AND-reduce over the 640 winner masks, mask = ~intersection.

Rationale: collectives on this 8-core setup have a hard ~60-90us cross-core
rendezvous floor (measured), so the global top-k merge is done on the host
from per-core partial results instead of with an on-device AllGather. The
device still does all the heavy lifting (the 4.3 GFLOP sims matmul, the
33M-element top-k scan, candidate row gather and product rows); the host
merge is O(640*512).

Rank-equivalences used (all verified numerically):
  - top-k over sims is invariant to the per-row scale 1/||t_b||.
  - bottom-256 membership of |g_n[j]*s_n[b]| equals that of |g_raw[j]*s_raw[b]|
    (row-wise positive scaling), so prod rows use raw g/s values, and bf16
    rounding of sims/prods only perturbs rank ties (the final AND-reduce over
    640 half-sets is insensitive to per-row rank slop).
"""

import sys

sys.path.insert(0, "/opt/trn_rl_repo")

import numpy as np

import concourse.bass as bass
import concourse.bacc as bacc
import concourse.mybir as mybir
import concourse.tile as tile
from concourse import bass_utils

B = 128
D = 512
N = 65536
NCORES = 8
NL = N // NCORES          # 8192 gallery rows per core
NT = NL // 512            # 16 column tiles of 512
KC = D // 128             # 4 contraction chunks
NC8 = 8                   # local top-k values kept per row (host merge input)
NG = 4                    # top slots whose gallery rows are gathered on device
                          # (a global winner ranks >4th within its own core only
                          # when one core holds >=5 of a row's global top-5,
                          # p ~ 2.4e-4/row; the host computes those rare rows
                          # directly from the inputs)

f32 = mybir.dt.float32
bf16 = mybir.dt.bfloat16
i16 = mybir.dt.int16
i32 = mybir.dt.int32
u16 = mybir.dt.uint16
Alu = mybir.AluOpType
AX = mybir.AxisListType


def build_program():
    nc = bacc.Bacc(
        "TRN2",
        target_bir_lowering=False,
        debug=False,
        num_devices=NCORES,
    )
    gnt = nc.dram_tensor("gnt", [NT, 128, KC * 512], bf16, kind="ExternalInput")
    grow = nc.dram_tensor("grow", [NL, D], bf16, kind="ExternalInput")
    tnt = nc.dram_tensor("tnt", [D, B], bf16, kind="ExternalInput")
    spre = nc.dram_tensor("spre", [128, NG, D], bf16, kind="ExternalInput")
    diag = nc.dram_tensor("diag", [B, B], f32, kind="ExternalInput")
    vals_out = nc.dram_tensor("vals", [B, NC8], f32, kind="ExternalOutput")
    idx_out = nc.dram_tensor("idx", [B, NC8], f32, kind="ExternalOutput")
    prod_out = nc.dram_tensor("prod", [128, NG, D], bf16, kind="ExternalOutput")

    with tile.TileContext(nc) as tc:
        _body(nc, tc, gnt, grow, tnt, spre, diag, vals_out, idx_out, prod_out)

    nc.compile()
    return nc


def _body(nc, tc, gnt, grow, tnt, spre, diag, vals_out, idx_out, prod_out):
    with (
        tc.tile_pool(name="const", bufs=1) as cp,
        tc.tile_pool(name="gstream", bufs=8) as gp,
        tc.tile_pool(name="psum", bufs=7, space="PSUM") as pp,
        tc.tile_pool(name="psum1", bufs=1, space="PSUM") as pp1,
        tc.tile_pool(name="work", bufs=1) as wp,
        tc.tile_pool(name="dram", bufs=1, space="DRAM") as dp,
    ):
        # ---- persistent SBUF tiles (only t_sb before the tile loop: every
        # other constant DMA is issued after it so tile 0's gallery DMA is not
        # delayed behind them in the queue)
        t_sb = cp.tile([128, KC, B], bf16)         # t_n.T as 4 contraction chunks
        nc.sync.dma_start(t_sb[:], tnt.rearrange("(k p) b -> p k b", k=KC))
        # static per-tile column offsets (0, 512, 1024, ...) as f32, built at
        # startup on idle engines so only the final add sits on the critical path
        offs = cp.tile([128, NT, 8], i32)
        nc.gpsimd.iota(offs[:], [[512, NT], [0, 8]], channel_multiplier=0)
        offsf = cp.tile([128, NT, 8], f32)
        nc.vector.tensor_copy(offsf[:], offs[:])

        # warm-up invocations of the gpsimd routines used in the tail, issued
        # while the engine is otherwise idle during phase A: the first call of
        # each routine pays a multi-us ucode-load/dispatch cost that would
        # otherwise sit on the critical path
        wz32 = cp.tile([128, 2], i32)
        nc.gpsimd.iota(wz32[:], [[0, 2]], channel_multiplier=0)
        wz16 = cp.tile([128, 2], i16)
        nc.vector.tensor_copy(wz16[:], wz32[:])
        wzu = cp.tile([128, 2], u16)
        nc.vector.tensor_copy(wzu[:], wz32[:])
        wzf = cp.tile([128, 2], f32)
        nc.vector.tensor_copy(wzf[:], wz32[:])
        wic = cp.tile([128, 2], f32)
        nc.gpsimd.indirect_copy(wic[:], wzf[:], wzu[:], True)
        wg = cp.tile([128, 1, D], bf16)
        nc.gpsimd.dma_gather(
            wg[:], grow.ap(), wz16[:, 0:1], 16, 16, D, single_packet=False
        )
        # PE warm-up: the PE runs ~1.7x slower for its first ~5us of activity
        # (clock ramp); burn that on dummies fed from memset tiles so the ramp
        # starts as early as possible, before any DMA lands
        wmm = cp.tile([128, 128], bf16)
        nc.vector.memset(wmm[:], 0.5)
        wrh = cp.tile([128, 512], bf16)
        nc.vector.memset(wrh[:], 0.5)
        wps = pp1.tile([128, 512], f32, tag="wps")
        for _ in range(8):
            nc.tensor.matmul(wps[:], lhsT=wmm[:], rhs=wrh[:], start=True, stop=True)
        wmax = cp.tile([128, 8], f32)
        nc.vector.max(wmax[:], wps[:])

        # ---- phase A: sims tiles + per-tile top-8 (vals + in-tile indices),
        # all overlapped with the gallery DMA stream
        cvals = wp.tile([128, NT, 8], f32)         # per-tile top-8 values
        cidx = wp.tile([128, NT, 8], u16)          # per-tile top-8 indices
        g_view = gnt.rearrange("t p (k j) -> t p k j", k=KC)
        for t in range(NT):
            gt = gp.tile([128, KC, 512], bf16, tag="gt")
            nc.sync.dma_start(gt[:], g_view[t])
            ps = pp.tile([128, 512], f32, tag="ps")
            for k in range(KC):
                nc.tensor.matmul(
                    ps[:],
                    lhsT=t_sb[:, k, :],
                    rhs=gt[:, k, :],
                    start=(k == 0),
                    stop=(k == KC - 1),
                )
            nc.vector.max(cvals[:, t, :], ps[:])
            nc.vector.max_index(cidx[:, t, :], cvals[:, t, :], ps[:])

        # diag for the A2 merge (needed only ~25us in, after the tile loop's
        # gallery DMAs have been queued)
        diag_sb = cp.tile([128, B], f32)           # diag[p, s*16+q] = (q == p%16)
        nc.sync.dma_start(diag_sb[:], diag.ap())

        # s_pre[p, g, :] = s_raw[8*(p%16) + 2g + p//64, :] matching the
        # gathered candidate at prod[p, g, :] (see the ordinal scheme at the
        # gather below). The permutation is pre-applied on the host so this is
        # a single contiguous DMA (8 strided DMAs here would clog the sync
        # queue right before the gather-critical idx bounce).
        s_pre = cp.tile([128, NG, D], bf16)
        nc.sync.dma_start(s_pre[:], spre.ap())

        # global-in-core candidate indices: in-tile index + 512*t, as f32
        cidxf = wp.tile([128, NT, 8], f32)
        nc.vector.tensor_copy(cidxf[:], cidx[:])
        gidxf = wp.tile([128, NT, 8], f32)
        nc.vector.tensor_tensor(gidxf[:], cidxf[:], offsf[:], Alu.add)

        # ---- phase A2: local top-8 over the 128 concatenated candidates
        lvals = wp.tile([128, 8], f32)
        pos = wp.tile([128, 8], u16)
        cvals2d = cvals[:].rearrange("p t s -> p (t s)")
        nc.vector.max(lvals[:], cvals2d)
        nc.vector.max_index(pos[:], lvals[:], cvals2d)
        # per-row gather gidxf[b, pos[b, s]] via group-shared indirect_copy:
        # out[p, s*16+q] = gidxf[p, pos[q_row, s]]; the diagonal q == p%16 is
        # each row's own positions -> mask by diag and sum over q.
        ic = wp.tile([128, B], f32)
        nc.gpsimd.indirect_copy(
            ic[:], gidxf[:].rearrange("p t s -> p (t s)"), pos[:], True
        )
        icm = wp.tile([128, 8, 16], f32)
        nc.vector.tensor_tensor(
            icm[:],
            ic[:].rearrange("p (s q) -> p s q", q=16),
            diag_sb[:].rearrange("p (s q) -> p s q", q=16),
            Alu.mult,
        )
        posf = wp.tile([128, 8], f32)
        nc.vector.tensor_reduce(posf[:], icm[:], axis=AX.X, op=Alu.add)

        # ship candidate values + local indices to the host merge (early, so
        # their transfers hide under the gather dispatch instead of trailing
        # the final output)
        nc.sync.dma_start(vals_out.ap(), lvals[:])
        nc.sync.dma_start(idx_out.ap(), posf[:])

        # ---- phase C: gather the top-NG candidate gallery rows, form products
        # Bounce the idx list (slots 0..NG-1) through DRAM (plain, flat
        # position z = 4b+s) and read it back wrapped+replicated with
        # idxw[p, f] = flat[32*(p%16)+f]. Under the gather's wrapped-16
        # semantics (ordinal r at partition r%16, slot r//16) this assigns
        # candidate z the ordinal r = 16*(z%32) + z//32, so the gathered row
        # at out[p, g] (ordinal g*128+p) is candidate b = 8*(p%16)+2g+p//64,
        # s = (p//16)%4 -- which is exactly the s_pre layout above. The host
        # undoes the same bijection.
        lidxi = wp.tile([128, NC8], i16)
        nc.vector.tensor_copy(lidxi[:], posf[:])
        ld = dp.tile([128, NG], i16)
        nc.sync.dma_start(ld[:], lidxi[:, 0:NG])
        idxw = wp.tile([128, NG * 8], i16)
        nc.sync.dma_start(
            idxw[:],
            bass.AP(ld.tensor, ld.offset, [[0, 8], [NG * 8, 16], [1, NG * 8]]),
        )

        # two half-gathers, each half's product and output DMA overlapping the
        # other half's gather
        grows = wp.tile([128, NG, D], bf16)
        prod = wp.tile([128, NG, D], bf16)
        H = NG // 2
        for h in range(2):
            gs = slice(h * H, (h + 1) * H)
            nc.gpsimd.dma_gather(
                grows[:, gs, :],
                grow.ap(),
                idxw[:, h * 16 : (h + 1) * 16],
                B * NG // 2,
                B * NG // 2,
                D,
                single_packet=False,
            )
            nc.vector.tensor_tensor(
                prod[:, gs, :], grows[:, gs, :], s_pre[:, gs, :], Alu.mult
            )
            # quarter-granularity output DMAs: the last transfer (which gates
            # kernel completion) shrinks to 128KB
            for g in range(h * H, (h + 1) * H):
                nc.sync.dma_start(prod_out.ap()[:, g : g + 1, :], prod[:, g : g + 1, :])


def _install_ntff_hook():
    """Recreate the antenv.axon_hooks NTFF profile hook this image lacks.

    bass_utils.run_bass_kernel_spmd(trace=True) imports
    antenv.axon_hooks.get_axon_ntff_profile_hook; the axon boot script on this
    image degraded silently because the module is absent. The hook is a thin
    ctypes wrapper over libaxon_pjrt.so's start/stop profile entry points.
    """
    import types, ctypes, contextlib

    if "antenv.axon_hooks" in sys.modules:
        return
    so_path = "/opt/axon/libaxon_pjrt.so"
    try:
        lib = ctypes.CDLL(so_path)
    except OSError:
        return
    if not hasattr(lib, "axon_start_nrt_profile"):
        return
    lib.axon_start_nrt_profile.argtypes = [
        ctypes.POINTER(ctypes.c_int64),
        ctypes.c_size_t,
    ]
    lib.axon_start_nrt_profile.restype = ctypes.c_int64
    lib.axon_stop_nrt_profile.argtypes = [ctypes.c_char_p]
    lib.axon_stop_nrt_profile.restype = ctypes.c_int64

    @contextlib.contextmanager
    def _hook(output_dir, device_ids):
        import jax

        jax.devices()
        if device_ids:
            ids = (ctypes.c_int64 * len(device_ids))(*device_ids)
            rc = lib.axon_start_nrt_profile(ids, len(device_ids))
        else:
            rc = lib.axon_start_nrt_profile(None, 0)
        if rc != 0:
            raise RuntimeError(f"axon_start_nrt_profile rc={rc}")
        try:
            yield
        finally:
            n = lib.axon_stop_nrt_profile(str(output_dir).encode())
            print(f"profile: {n} file(s) written to {output_dir}", file=sys.stderr)

    mod = types.ModuleType("antenv.axon_hooks")
    _state = {"hook": _hook}
    mod.get_axon_ntff_profile_hook = lambda: _state["hook"]
    mod.set_axon_ntff_profile_hook = lambda h: _state.__setitem__("hook", h)
    sys.modules["antenv.axon_hooks"] = mod
    import antenv

    antenv.axon_hooks = mod


_PROGRAM = None


def _get_program():
    global _PROGRAM
    if _PROGRAM is None:
        _PROGRAM = build_program()
    return _PROGRAM


def _prep_inputs(s_f, t_f, gallery):
    import ml_dtypes

    s_f = np.ascontiguousarray(np.asarray(s_f, dtype=np.float32))
    t_f = np.ascontiguousarray(np.asarray(t_f, dtype=np.float32))
    gallery = np.ascontiguousarray(np.asarray(gallery, dtype=np.float32))

    tn = t_f / np.maximum(np.linalg.norm(t_f, axis=1, keepdims=True), 1e-12)
    gn = gallery / np.maximum(np.linalg.norm(gallery, axis=1, keepdims=True), 1e-12)
    tnt = np.ascontiguousarray(tn.T).astype(ml_dtypes.bfloat16)
    sf16 = s_f.astype(ml_dtypes.bfloat16)

    # s rows pre-permuted to the gather's output layout:
    # spre[p, g, :] = s[8*(p%16) + 2g + p//64, :]
    pp = np.arange(128)
    gg = np.arange(NG)
    bmap = 8 * (pp[:, None] % 16) + 2 * gg[None, :] + pp[:, None] // 64
    spre = np.ascontiguousarray(sf16[bmap])

    p = np.arange(B)[:, None]
    i = np.arange(B)[None, :]
    diag = ((i % 16) == (p % 16)).astype(np.float32)
    in_maps = []
    for c in range(NCORES):
        sl = slice(c * NL, (c + 1) * NL)
        # [D, NL] -> [NT, 128, KC*512]: tile t's DMA reads 4KB contiguous rows
        gsh = gn[sl].T.reshape(KC, 128, NT, 512)
        gsh = np.ascontiguousarray(gsh.transpose(2, 1, 0, 3)).reshape(NT, 128, KC * 512)
        in_maps.append(
            {
                "gnt": gsh.astype(ml_dtypes.bfloat16),
                "grow": np.ascontiguousarray(gallery[sl]).astype(ml_dtypes.bfloat16),
                "tnt": tnt,
                "spre": spre,
                "diag": diag,
            }
        )
    return in_maps


def kernel(s_f, t_f, gallery, _trace=False):
    if _trace:
        _install_ntff_hook()
    nc = _get_program()
    in_maps = _prep_inputs(s_f, t_f, gallery)
    res = bass_utils.run_bass_kernel_spmd(
        nc, in_maps, core_ids=list(range(NCORES)), trace=_trace
    )

    # host merge: global top-5 per row over the 8 cores' local top-8 lists
    vals = np.stack(
        [res.results[c]["vals"].reshape(B, NC8) for c in range(NCORES)]
    )  # [8, B, 8]
    prods = np.stack(
        [
            np.asarray(res.results[c]["prod"]).astype(np.float32)
            for c in range(NCORES)
        ]
    )  # [8, 128, NG, D]

    v = vals.transpose(1, 0, 2).reshape(B, NCORES * NC8)  # [B, 64]
    top5 = np.argpartition(-v, 5, axis=1)[:, :5]          # [B, 5] flat (core, slot)

    bb = np.arange(B)
    core = top5 // NC8                                    # [B, 5]
    slot = top5 % NC8
    # candidate (b, s<NG) was gathered to prod[64*((b%8)%2) + 16*s + b//8,
    # (b%8)//2, :] on its core
    sl = np.minimum(slot, NG - 1)
    p = 64 * ((bb[:, None] % 8) % 2) + 16 * sl + bb[:, None] // 8
    g = (bb[:, None] % 8) // 2
    drows = prods[core, p, g]                             # [B, 5, D]
    d = np.abs(drows)

    # rare fallback: a winner that ranked >NG-th inside its own core (one core
    # held >=5 of that row's global top-5) was not gathered on device --
    # compute its product row from the staged inputs directly
    rare_b, rare_k = np.nonzero(slot >= NG)
    if rare_b.size:
        idxs = np.stack(
            [res.results[c]["idx"].reshape(B, NC8) for c in range(NCORES)]
        )  # [8, B, 8] local gallery indices as f32
        for b, k in zip(rare_b, rare_k):
            c, s = core[b, k], slot[b, k]
            j = int(idxs[c, b, s]) + c * NL
            d[b, k] = np.abs(gallery[j] * s_f[b])

    m = D // 2
    low_idx = np.argpartition(d, m, axis=-1)[..., :m]     # [B, 5, m]
    member = np.zeros((B, 5, D), dtype=bool)
    member[bb[:, None, None], np.arange(5)[None, :, None], low_idx] = True
    zero_out = member.all(axis=(0, 1))
    mask = np.where(zero_out, 0.0, 1.0).astype(np.float32)

    if _trace:
        kernel.last_exec_time_ns = res.exec_time_ns
        kernel.last_results = res
    return mask


# revision 52
# speedup vs baseline: 1.1312x; 1.1312x over previous
"""Bass/Trainium2 kernel for nn_Context_RGR_20718922235945 (retrieval_knn).

No-collective design (8 NeuronCores, gallery sharded along N):
  host : normalize t and gallery, cast to bf16, transpose for DMA-friendly
         layouts, shard gallery N across 8 cores.
  core : sims slab  [128, 8192] = t_n @ g_n_shard.T   (PE, bf16 -> f32 psum)
         local top-8 values+indices per batch row      (DVE max / max_index)
         gather all 1024 candidate gallery rows        (gpsimd dma_gather)
         prod rows = g_row * s_row (elementwise)       (DVE)
         DMA out: top-8 values, top-8 local indices, 1024 prod rows (bf16)
  host : merge the 8 local top-8 lists -> global top-5 per row (the
         gather/unshard "reduce" step), bottom-256 membership per winner row,
         AND-reduce over the 640 winner masks, mask = ~intersection.

Rationale: collectives on this 8-core setup have a hard ~60-90us cross-core
rendezvous floor (measured), so the global top-k merge is done on the host
from per-core partial results instead of with an on-device AllGather. The
device still does all the heavy lifting (the 4.3 GFLOP sims matmul, the
33M-element top-k scan, candidate row gather and product rows); the host
merge is O(640*512).

Rank-equivalences used (all verified numerically):
  - top-k over sims is invariant to the per-row scale 1/||t_b||.
  - bottom-256 membership of |g_n[j]*s_n[b]| equals that of |g_raw[j]*s_raw[b]|
    (row-wise positive scaling), so prod rows use raw g/s values, and bf16
    rounding of sims/prods only perturbs rank ties (the final AND-reduce over
    640 half-sets is insensitive to per-row rank slop).
"""

import sys

sys.path.insert(0, "/opt/trn_rl_repo")

import numpy as np

import concourse.bass as bass
import concourse.bacc as bacc
import concourse.mybir as mybir
import concourse.tile as tile
from concourse import bass_utils

B = 128
D = 512
N = 65536
NCORES = 8
NL = N // NCORES          # 8192 gallery rows per core
NT = NL // 512            # 16 column tiles of 512
KC = D // 128             # 4 contraction chunks
NC8 = 8                   # local top-k values kept per row (host merge input)
NG = 4                    # top slots whose gallery rows are gathered on device
                          # (a global winner ranks >4th within its own core only
                          # when one core holds >=5 of a row's global top-5,
                          # p ~ 2.4e-4/row; the host computes those rare rows
                          # directly from the inputs)

f32 = mybir.dt.float32
bf16 = mybir.dt.bfloat16
i16 = mybir.dt.int16
i32 = mybir.dt.int32
u16 = mybir.dt.uint16
Alu = mybir.AluOpType
AX = mybir.AxisListType


def build_program():
    nc = bacc.Bacc(
        "TRN2",
        target_bir_lowering=False,
        debug=False,
        num_devices=NCORES,
    )
    gnt = nc.dram_tensor("gnt", [NT, 128, KC * 512], bf16, kind="ExternalInput")
    grow = nc.dram_tensor("grow", [NL, D], bf16, kind="ExternalInput")
    tnt = nc.dram_tensor("tnt", [D, B], bf16, kind="ExternalInput")
    spre = nc.dram_tensor("spre", [128, NG, D], bf16, kind="ExternalInput")
    diag = nc.dram_tensor("diag", [B, B], f32, kind="ExternalInput")
    vals_out = nc.dram_tensor("vals", [B, NC8], f32, kind="ExternalOutput")
    idx_out = nc.dram_tensor("idx", [B, NC8], f32, kind="ExternalOutput")
    prod_out = nc.dram_tensor("prod", [128, NG, D], bf16, kind="ExternalOutput")

    with tile.TileContext(nc) as tc:
        _body(nc, tc, gnt, grow, tnt, spre, diag, vals_out, idx_out, prod_out)

    nc.compile()
    return nc


def _body(nc, tc, gnt, grow, tnt, spre, diag, vals_out, idx_out, prod_out):
    with (
        tc.tile_pool(name="const", bufs=1) as cp,
        tc.tile_pool(name="gstream", bufs=6) as gp,
        tc.tile_pool(name="psum", bufs=6, space="PSUM") as pp,
        tc.tile_pool(name="psum1", bufs=1, space="PSUM") as pp1,
        tc.tile_pool(name="work", bufs=1) as wp,
        tc.tile_pool(name="dram", bufs=1, space="DRAM") as dp,
    ):
        # ---- persistent SBUF tiles (only t_sb before the tile loop: every
        # other constant DMA is issued after it so tile 0's gallery DMA is not
        # delayed behind them in the queue)
        t_sb = cp.tile([128, KC, B], bf16)         # t_n.T as 4 contraction chunks
        nc.sync.dma_start(t_sb[:], tnt.rearrange("(k p) b -> p k b", k=KC))
        # static per-tile column offsets (0, 512, 1024, ...) as f32, built at
        # startup on idle engines so only the final add sits on the critical path
        offs = cp.tile([128, NT, 8], i32)
        nc.gpsimd.iota(offs[:], [[512, NT], [0, 8]], channel_multiplier=0)
        offsf = cp.tile([128, NT, 8], f32)
        nc.vector.tensor_copy(offsf[:], offs[:])

        # warm-up invocations of the gpsimd routines used in the tail, issued
        # while the engine is otherwise idle during phase A: the first call of
        # each routine pays a multi-us ucode-load/dispatch cost that would
        # otherwise sit on the critical path
        wz32 = cp.tile([128, 2], i32)
        nc.gpsimd.iota(wz32[:], [[0, 2]], channel_multiplier=0)
        wz16 = cp.tile([128, 2], i16)
        nc.vector.tensor_copy(wz16[:], wz32[:])
        wzu = cp.tile([128, 2], u16)
        nc.vector.tensor_copy(wzu[:], wz32[:])
        wzf = cp.tile([128, 2], f32)
        nc.vector.tensor_copy(wzf[:], wz32[:])
        wic = cp.tile([128, 2], f32)
        nc.gpsimd.indirect_copy(wic[:], wzf[:], wzu[:], True)
        wg = cp.tile([128, 1, D], bf16)
        nc.gpsimd.dma_gather(
            wg[:], grow.ap(), wz16[:, 0:1], 16, 16, D, single_packet=False
        )
        # PE warm-up: the first ~6 matmuls after idle run ~1.7x slower (clock
        # ramp); burn that on dummies against t_sb while tile 0's gallery DMA
        # is still in flight
        wps = pp1.tile([128, 512], f32, tag="wps")
        t_flat = t_sb[:].rearrange("p k b -> p (k b)")
        for _ in range(6):
            nc.tensor.matmul(wps[:], lhsT=t_sb[:, 0, :], rhs=t_flat, start=True, stop=True)
        wmax = cp.tile([128, 8], f32)
        nc.vector.max(wmax[:], wps[:])

        # ---- phase A: sims tiles + per-tile top-8 (vals + in-tile indices),
        # all overlapped with the gallery DMA stream
        cvals = wp.tile([128, NT, 8], f32)         # per-tile top-8 values
        cidx = wp.tile([128, NT, 8], u16)          # per-tile top-8 indices
        g_view = gnt.rearrange("t p (k j) -> t p k j", k=KC)
        for t in range(NT):
            gt = gp.tile([128, KC, 512], bf16, tag="gt")
            nc.sync.dma_start(gt[:], g_view[t])
            ps = pp.tile([128, 512], f32, tag="ps")
            for k in range(KC):
                nc.tensor.matmul(
                    ps[:],
                    lhsT=t_sb[:, k, :],
                    rhs=gt[:, k, :],
                    start=(k == 0),
                    stop=(k == KC - 1),
                )
            nc.vector.max(cvals[:, t, :], ps[:])
            nc.vector.max_index(cidx[:, t, :], cvals[:, t, :], ps[:])

        # diag for the A2 merge (needed only ~25us in, after the tile loop's
        # gallery DMAs have been queued)
        diag_sb = cp.tile([128, B], f32)           # diag[p, s*16+q] = (q == p%16)
        nc.sync.dma_start(diag_sb[:], diag.ap())

        # s_pre[p, g, :] = s_raw[8*(p%16) + 2g + p//64, :] matching the
        # gathered candidate at prod[p, g, :] (see the ordinal scheme at the
        # gather below). The permutation is pre-applied on the host so this is
        # a single contiguous DMA (8 strided DMAs here would clog the sync
        # queue right before the gather-critical idx bounce).
        s_pre = cp.tile([128, NG, D], bf16)
        nc.sync.dma_start(s_pre[:], spre.ap())

        # global-in-core candidate indices: in-tile index + 512*t, as f32
        cidxf = wp.tile([128, NT, 8], f32)
        nc.vector.tensor_copy(cidxf[:], cidx[:])
        gidxf = wp.tile([128, NT, 8], f32)
        nc.vector.tensor_tensor(gidxf[:], cidxf[:], offsf[:], Alu.add)

        # ---- phase A2: local top-8 over the 128 concatenated candidates
        lvals = wp.tile([128, 8], f32)
        pos = wp.tile([128, 8], u16)
        cvals2d = cvals[:].rearrange("p t s -> p (t s)")
        nc.vector.max(lvals[:], cvals2d)
        nc.vector.max_index(pos[:], lvals[:], cvals2d)
        # per-row gather gidxf[b, pos[b, s]] via group-shared indirect_copy:
        # out[p, s*16+q] = gidxf[p, pos[q_row, s]]; the diagonal q == p%16 is
        # each row's own positions -> mask by diag and sum over q.
        ic = wp.tile([128, B], f32)
        nc.gpsimd.indirect_copy(
            ic[:], gidxf[:].rearrange("p t s -> p (t s)"), pos[:], True
        )
        icm = wp.tile([128, 8, 16], f32)
        nc.vector.tensor_tensor(
            icm[:],
            ic[:].rearrange("p (s q) -> p s q", q=16),
            diag_sb[:].rearrange("p (s q) -> p s q", q=16),
            Alu.mult,
        )
        posf = wp.tile([128, 8], f32)
        nc.vector.tensor_reduce(posf[:], icm[:], axis=AX.X, op=Alu.add)

        # ship candidate values + local indices to the host merge (early, so
        # their transfers hide under the gather dispatch instead of trailing
        # the final output)
        nc.sync.dma_start(vals_out.ap(), lvals[:])
        nc.sync.dma_start(idx_out.ap(), posf[:])

        # ---- phase C: gather the top-NG candidate gallery rows, form products
        # Bounce the idx list (slots 0..NG-1) through DRAM (plain, flat
        # position z = 4b+s) and read it back wrapped+replicated with
        # idxw[p, f] = flat[32*(p%16)+f]. Under the gather's wrapped-16
        # semantics (ordinal r at partition r%16, slot r//16) this assigns
        # candidate z the ordinal r = 16*(z%32) + z//32, so the gathered row
        # at out[p, g] (ordinal g*128+p) is candidate b = 8*(p%16)+2g+p//64,
        # s = (p//16)%4 -- which is exactly the s_pre layout above. The host
        # undoes the same bijection.
        lidxi = wp.tile([128, NC8], i16)
        nc.vector.tensor_copy(lidxi[:], posf[:])
        ld = dp.tile([128, NG], i16)
        nc.sync.dma_start(ld[:], lidxi[:, 0:NG])
        idxw = wp.tile([128, NG * 8], i16)
        nc.sync.dma_start(
            idxw[:],
            bass.AP(ld.tensor, ld.offset, [[0, 8], [NG * 8, 16], [1, NG * 8]]),
        )

        # two half-gathers, each half's product and output DMA overlapping the
        # other half's gather
        grows = wp.tile([128, NG, D], bf16)
        prod = wp.tile([128, NG, D], bf16)
        H = NG // 2
        for h in range(2):
            gs = slice(h * H, (h + 1) * H)
            nc.gpsimd.dma_gather(
                grows[:, gs, :],
                grow.ap(),
                idxw[:, h * 16 : (h + 1) * 16],
                B * NG // 2,
                B * NG // 2,
                D,
                single_packet=False,
            )
            nc.vector.tensor_tensor(
                prod[:, gs, :], grows[:, gs, :], s_pre[:, gs, :], Alu.mult
            )
            nc.sync.dma_start(prod_out.ap()[:, gs, :], prod[:, gs, :])


def _install_ntff_hook():
    """Recreate the antenv.axon_hooks NTFF profile hook this image lacks.

    bass_utils.run_bass_kernel_spmd(trace=True) imports
    antenv.axon_hooks.get_axon_ntff_profile_hook; the axon boot script on this
    image degraded silently because the module is absent. The hook is a thin
    ctypes wrapper over libaxon_pjrt.so's start/stop profile entry points.
    """
    import types, ctypes, contextlib

    if "antenv.axon_hooks" in sys.modules:
        return
    so_path = "/opt/axon/libaxon_pjrt.so"
    try:
        lib = ctypes.CDLL(so_path)
    except OSError:
        return
    if not hasattr(lib, "axon_start_nrt_profile"):
        return
    lib.axon_start_nrt_profile.argtypes = [
        ctypes.POINTER(ctypes.c_int64),
        ctypes.c_size_t,
    ]
    lib.axon_start_nrt_profile.restype = ctypes.c_int64
    lib.axon_stop_nrt_profile.argtypes = [ctypes.c_char_p]
    lib.axon_stop_nrt_profile.restype = ctypes.c_int64

    @contextlib.contextmanager
    def _hook(output_dir, device_ids):
        import jax

        jax.devices()
        if device_ids:
            ids = (ctypes.c_int64 * len(device_ids))(*device_ids)
            rc = lib.axon_start_nrt_profile(ids, len(device_ids))
        else:
            rc = lib.axon_start_nrt_profile(None, 0)
        if rc != 0:
            raise RuntimeError(f"axon_start_nrt_profile rc={rc}")
        try:
            yield
        finally:
            n = lib.axon_stop_nrt_profile(str(output_dir).encode())
            print(f"profile: {n} file(s) written to {output_dir}", file=sys.stderr)

    mod = types.ModuleType("antenv.axon_hooks")
    _state = {"hook": _hook}
    mod.get_axon_ntff_profile_hook = lambda: _state["hook"]
    mod.set_axon_ntff_profile_hook = lambda h: _state.__setitem__("hook", h)
    sys.modules["antenv.axon_hooks"] = mod
    import antenv

    antenv.axon_hooks = mod


_PROGRAM = None


def _get_program():
    global _PROGRAM
    if _PROGRAM is None:
        _PROGRAM = build_program()
    return _PROGRAM


def _prep_inputs(s_f, t_f, gallery):
    import ml_dtypes

    s_f = np.ascontiguousarray(np.asarray(s_f, dtype=np.float32))
    t_f = np.ascontiguousarray(np.asarray(t_f, dtype=np.float32))
    gallery = np.ascontiguousarray(np.asarray(gallery, dtype=np.float32))

    tn = t_f / np.maximum(np.linalg.norm(t_f, axis=1, keepdims=True), 1e-12)
    gn = gallery / np.maximum(np.linalg.norm(gallery, axis=1, keepdims=True), 1e-12)
    tnt = np.ascontiguousarray(tn.T).astype(ml_dtypes.bfloat16)
    sf16 = s_f.astype(ml_dtypes.bfloat16)

    # s rows pre-permuted to the gather's output layout:
    # spre[p, g, :] = s[8*(p%16) + 2g + p//64, :]
    pp = np.arange(128)
    gg = np.arange(NG)
    bmap = 8 * (pp[:, None] % 16) + 2 * gg[None, :] + pp[:, None] // 64
    spre = np.ascontiguousarray(sf16[bmap])

    p = np.arange(B)[:, None]
    i = np.arange(B)[None, :]
    diag = ((i % 16) == (p % 16)).astype(np.float32)
    in_maps = []
    for c in range(NCORES):
        sl = slice(c * NL, (c + 1) * NL)
        # [D, NL] -> [NT, 128, KC*512]: tile t's DMA reads 4KB contiguous rows
        gsh = gn[sl].T.reshape(KC, 128, NT, 512)
        gsh = np.ascontiguousarray(gsh.transpose(2, 1, 0, 3)).reshape(NT, 128, KC * 512)
        in_maps.append(
            {
                "gnt": gsh.astype(ml_dtypes.bfloat16),
                "grow": np.ascontiguousarray(gallery[sl]).astype(ml_dtypes.bfloat16),
                "tnt": tnt,
                "spre": spre,
                "diag": diag,
            }
        )
    return in_maps


def kernel(s_f, t_f, gallery, _trace=False):
    if _trace:
        _install_ntff_hook()
    nc = _get_program()
    in_maps = _prep_inputs(s_f, t_f, gallery)
    res = bass_utils.run_bass_kernel_spmd(
        nc, in_maps, core_ids=list(range(NCORES)), trace=_trace
    )

    # host merge: global top-5 per row over the 8 cores' local top-8 lists
    vals = np.stack(
        [res.results[c]["vals"].reshape(B, NC8) for c in range(NCORES)]
    )  # [8, B, 8]
    prods = np.stack(
        [
            np.asarray(res.results[c]["prod"]).astype(np.float32)
            for c in range(NCORES)
        ]
    )  # [8, 128, NG, D]

    v = vals.transpose(1, 0, 2).reshape(B, NCORES * NC8)  # [B, 64]
    top5 = np.argpartition(-v, 5, axis=1)[:, :5]          # [B, 5] flat (core, slot)

    bb = np.arange(B)
    core = top5 // NC8                                    # [B, 5]
    slot = top5 % NC8
    # candidate (b, s<NG) was gathered to prod[64*((b%8)%2) + 16*s + b//8,
    # (b%8)//2, :] on its core
    sl = np.minimum(slot, NG - 1)
    p = 64 * ((bb[:, None] % 8) % 2) + 16 * sl + bb[:, None] // 8
    g = (bb[:, None] % 8) // 2
    drows = prods[core, p, g]                             # [B, 5, D]
    d = np.abs(drows)

    # rare fallback: a winner that ranked >NG-th inside its own core (one core
    # held >=5 of that row's global top-5) was not gathered on device --
    # compute its product row from the staged inputs directly
    rare_b, rare_k = np.nonzero(slot >= NG)
    if rare_b.size:
        idxs = np.stack(
            [res.results[c]["idx"].reshape(B, NC8) for c in range(NCORES)]
        )  # [8, B, 8] local gallery indices as f32
        for b, k in zip(rare_b, rare_k):
            c, s = core[b, k], slot[b, k]
            j = int(idxs[c, b, s]) + c * NL
            d[b, k] = np.abs(gallery[j] * s_f[b])

    m = D // 2
    low_idx = np.argpartition(d, m, axis=-1)[..., :m]     # [B, 5, m]
    member = np.zeros((B, 5, D), dtype=bool)
    member[bb[:, None, None], np.arange(5)[None, :, None], low_idx] = True
    zero_out = member.all(axis=(0, 1))
    mask = np.where(zero_out, 0.0, 1.0).astype(np.float32)

    if _trace:
        kernel.last_exec_time_ns = res.exec_time_ns
        kernel.last_results = res
    return mask


# revision 53
# speedup vs baseline: 1.1833x; 1.0460x over previous
"""Bass/Trainium2 kernel for nn_Context_RGR_20718922235945 (retrieval_knn).

No-collective design (8 NeuronCores, gallery sharded along N):
  host : normalize t and gallery, cast to bf16, transpose for DMA-friendly
         layouts, shard gallery N across 8 cores.
  core : sims slab  [128, 8192] = t_n @ g_n_shard.T   (PE, bf16 -> f32 psum)
         local top-8 values+indices per batch row      (DVE max / max_index)
         gather all 1024 candidate gallery rows        (gpsimd dma_gather)
         prod rows = g_row * s_row (elementwise)       (DVE)
         DMA out: top-8 values, top-8 local indices, 1024 prod rows (bf16)
  host : merge the 8 local top-8 lists -> global top-5 per row (the
         gather/unshard "reduce" step), bottom-256 membership per winner row,
         AND-reduce over the 640 winner masks, mask = ~intersection.

Rationale: collectives on this 8-core setup have a hard ~60-90us cross-core
rendezvous floor (measured), so the global top-k merge is done on the host
from per-core partial results instead of with an on-device AllGather. The
device still does all the heavy lifting (the 4.3 GFLOP sims matmul, the
33M-element top-k scan, candidate row gather and product rows); the host
merge is O(640*512).

Rank-equivalences used (all verified numerically):
  - top-k over sims is invariant to the per-row scale 1/||t_b||.
  - bottom-256 membership of |g_n[j]*s_n[b]| equals that of |g_raw[j]*s_raw[b]|
    (row-wise positive scaling), so prod rows use raw g/s values, and bf16
    rounding of sims/prods only perturbs rank ties (the final AND-reduce over
    640 half-sets is insensitive to per-row rank slop).
"""

import sys

sys.path.insert(0, "/opt/trn_rl_repo")

import numpy as np

import concourse.bass as bass
import concourse.bacc as bacc
import concourse.mybir as mybir
import concourse.tile as tile
from concourse import bass_utils

B = 128
D = 512
N = 65536
NCORES = 8
NL = N // NCORES          # 8192 gallery rows per core
NT = NL // 512            # 16 column tiles of 512
KC = D // 128             # 4 contraction chunks
NC8 = 8                   # local top-k values kept per row (host merge input)
NG = 4                    # top slots whose gallery rows are gathered on device
                          # (a global winner ranks >4th within its own core only
                          # when one core holds >=5 of a row's global top-5,
                          # p ~ 2.4e-4/row; the host computes those rare rows
                          # directly from the inputs)

f32 = mybir.dt.float32
bf16 = mybir.dt.bfloat16
f8 = mybir.dt.float8e4
i16 = mybir.dt.int16
i32 = mybir.dt.int32
u16 = mybir.dt.uint16
Alu = mybir.AluOpType
AX = mybir.AxisListType


def build_program():
    nc = bacc.Bacc(
        "TRN2",
        target_bir_lowering=False,
        debug=False,
        num_devices=NCORES,
    )
    gnt = nc.dram_tensor("gnt", [NT, 128, KC * 512], f8, kind="ExternalInput")
    grow = nc.dram_tensor("grow", [NL, D], bf16, kind="ExternalInput")
    tnt = nc.dram_tensor("tnt", [D, B], f8, kind="ExternalInput")
    spre = nc.dram_tensor("spre", [128, NG, D], bf16, kind="ExternalInput")
    diag = nc.dram_tensor("diag", [B, B], f32, kind="ExternalInput")
    vals_out = nc.dram_tensor("vals", [B, NC8], f32, kind="ExternalOutput")
    idx_out = nc.dram_tensor("idx", [B, NC8], f32, kind="ExternalOutput")
    prod_out = nc.dram_tensor("prod", [128, NG, D], bf16, kind="ExternalOutput")

    with tile.TileContext(nc) as tc:
        _body(nc, tc, gnt, grow, tnt, spre, diag, vals_out, idx_out, prod_out)

    nc.compile()
    return nc


def _body(nc, tc, gnt, grow, tnt, spre, diag, vals_out, idx_out, prod_out):
    with (
        tc.tile_pool(name="const", bufs=1) as cp,
        tc.tile_pool(name="gstream", bufs=6) as gp,
        tc.tile_pool(name="psum", bufs=6, space="PSUM") as pp,
        tc.tile_pool(name="psum1", bufs=1, space="PSUM") as pp1,
        tc.tile_pool(name="work", bufs=1) as wp,
        tc.tile_pool(name="dram", bufs=1, space="DRAM") as dp,
    ):
        # ---- persistent SBUF tiles (only t_sb before the tile loop: every
        # other constant DMA is issued after it so tile 0's gallery DMA is not
        # delayed behind them in the queue)
        t_sb = cp.tile([128, KC, B], f8)           # t_n.T as 4 contraction chunks
        nc.sync.dma_start(t_sb[:], tnt.rearrange("(k p) b -> p k b", k=KC))
        # static per-tile column offsets (0, 512, 1024, ...) as f32, built at
        # startup on idle engines so only the final add sits on the critical path
        offs = cp.tile([128, NT, 8], i32)
        nc.gpsimd.iota(offs[:], [[512, NT], [0, 8]], channel_multiplier=0)
        offsf = cp.tile([128, NT, 8], f32)
        nc.vector.tensor_copy(offsf[:], offs[:])

        # warm-up invocations of the gpsimd routines used in the tail, issued
        # while the engine is otherwise idle during phase A: the first call of
        # each routine pays a multi-us ucode-load/dispatch cost that would
        # otherwise sit on the critical path
        wz32 = cp.tile([128, 2], i32)
        nc.gpsimd.iota(wz32[:], [[0, 2]], channel_multiplier=0)
        wz16 = cp.tile([128, 2], i16)
        nc.vector.tensor_copy(wz16[:], wz32[:])
        wzu = cp.tile([128, 2], u16)
        nc.vector.tensor_copy(wzu[:], wz32[:])
        wzf = cp.tile([128, 2], f32)
        nc.vector.tensor_copy(wzf[:], wz32[:])
        wic = cp.tile([128, 2], f32)
        nc.gpsimd.indirect_copy(wic[:], wzf[:], wzu[:], True)
        wg = cp.tile([128, 1, D], bf16)
        nc.gpsimd.dma_gather(
            wg[:], grow.ap(), wz16[:, 0:1], 16, 16, D, single_packet=False
        )
        # PE warm-up: the first ~6 matmuls after idle run ~1.7x slower (clock
        # ramp); burn that on dummies against t_sb while tile 0's gallery DMA
        # is still in flight
        wps = pp1.tile([128, 512], f32, tag="wps")
        t_flat = t_sb[:].rearrange("p k b -> p (k b)")
        for _ in range(6):
            nc.tensor.matmul(wps[:], lhsT=t_sb[:, 0, :], rhs=t_flat, start=True, stop=True)
        wmax = cp.tile([128, 8], f32)
        nc.vector.max(wmax[:], wps[:])

        # ---- phase A: sims tiles + per-tile top-8 (vals + in-tile indices),
        # all overlapped with the gallery DMA stream
        cvals = wp.tile([128, NT, 8], bf16)        # per-tile top-8 values
        cidx = wp.tile([128, NT, 8], u16)          # per-tile top-8 indices
        sims = wp.tile([128, NT, 512], bf16)       # bf16 sims (scalar-copied)
        g_view = gnt.rearrange("t p (k j) -> t p k j", k=KC)
        for t in range(NT):
            gt = gp.tile([128, KC, 512], f8, tag="gt")
            nc.sync.dma_start(gt[:], g_view[t])
            ps = pp.tile([128, 512], f32, tag="ps")
            for k in range(KC):
                nc.tensor.matmul(
                    ps[:],
                    lhsT=t_sb[:, k, :],
                    rhs=gt[:, k, :],
                    start=(k == 0),
                    stop=(k == KC - 1),
                )
            # scalar engine downcasts the sims tile; both DVE scans then run
            # in bf16 (2 elem/cycle) so the vector engine stays off the
            # critical path at the fp8 DMA rate
            nc.scalar.copy(sims[:, t, :], ps[:])
            nc.vector.max(cvals[:, t, :], sims[:, t, :])
            nc.vector.max_index(cidx[:, t, :], cvals[:, t, :], sims[:, t, :])

        # diag for the A2 merge (needed only ~25us in, after the tile loop's
        # gallery DMAs have been queued)
        diag_sb = cp.tile([128, B], f32)           # diag[p, s*16+q] = (q == p%16)
        nc.sync.dma_start(diag_sb[:], diag.ap())

        # s_pre[p, g, :] = s_raw[8*(p%16) + 2g + p//64, :] matching the
        # gathered candidate at prod[p, g, :] (see the ordinal scheme at the
        # gather below). The permutation is pre-applied on the host so this is
        # a single contiguous DMA (8 strided DMAs here would clog the sync
        # queue right before the gather-critical idx bounce).
        s_pre = cp.tile([128, NG, D], bf16)
        nc.sync.dma_start(s_pre[:], spre.ap())

        # global-in-core candidate indices: in-tile index + 512*t, as f32
        cidxf = wp.tile([128, NT, 8], f32)
        nc.vector.tensor_copy(cidxf[:], cidx[:])
        gidxf = wp.tile([128, NT, 8], f32)
        nc.vector.tensor_tensor(gidxf[:], cidxf[:], offsf[:], Alu.add)

        # ---- phase A2: local top-8 over the 128 concatenated candidates
        lvals = wp.tile([128, 8], bf16)
        pos = wp.tile([128, 8], u16)
        cvals2d = cvals[:].rearrange("p t s -> p (t s)")
        nc.vector.max(lvals[:], cvals2d)
        nc.vector.max_index(pos[:], lvals[:], cvals2d)
        lvalsf = wp.tile([128, 8], f32)
        nc.vector.tensor_copy(lvalsf[:], lvals[:])
        # per-row gather gidxf[b, pos[b, s]] via group-shared indirect_copy:
        # out[p, s*16+q] = gidxf[p, pos[q_row, s]]; the diagonal q == p%16 is
        # each row's own positions -> mask by diag and sum over q.
        ic = wp.tile([128, B], f32)
        nc.gpsimd.indirect_copy(
            ic[:], gidxf[:].rearrange("p t s -> p (t s)"), pos[:], True
        )
        icm = wp.tile([128, 8, 16], f32)
        nc.vector.tensor_tensor(
            icm[:],
            ic[:].rearrange("p (s q) -> p s q", q=16),
            diag_sb[:].rearrange("p (s q) -> p s q", q=16),
            Alu.mult,
        )
        posf = wp.tile([128, 8], f32)
        nc.vector.tensor_reduce(posf[:], icm[:], axis=AX.X, op=Alu.add)

        # ship candidate values + local indices to the host merge (early, so
        # their transfers hide under the gather dispatch instead of trailing
        # the final output)
        nc.sync.dma_start(vals_out.ap(), lvalsf[:])
        nc.sync.dma_start(idx_out.ap(), posf[:])

        # ---- phase C: gather the top-NG candidate gallery rows, form products
        # Bounce the idx list (slots 0..NG-1) through DRAM (plain, flat
        # position z = 4b+s) and read it back wrapped+replicated with
        # idxw[p, f] = flat[32*(p%16)+f]. Under the gather's wrapped-16
        # semantics (ordinal r at partition r%16, slot r//16) this assigns
        # candidate z the ordinal r = 16*(z%32) + z//32, so the gathered row
        # at out[p, g] (ordinal g*128+p) is candidate b = 8*(p%16)+2g+p//64,
        # s = (p//16)%4 -- which is exactly the s_pre layout above. The host
        # undoes the same bijection.
        lidxi = wp.tile([128, NC8], i16)
        nc.vector.tensor_copy(lidxi[:], posf[:])
        ld = dp.tile([128, NG], i16)
        nc.sync.dma_start(ld[:], lidxi[:, 0:NG])
        idxw = wp.tile([128, NG * 8], i16)
        nc.sync.dma_start(
            idxw[:],
            bass.AP(ld.tensor, ld.offset, [[0, 8], [NG * 8, 16], [1, NG * 8]]),
        )

        # two half-gathers, each half's product and output DMA overlapping the
        # other half's gather
        grows = wp.tile([128, NG, D], bf16)
        prod = wp.tile([128, NG, D], bf16)
        H = NG // 2
        for h in range(2):
            gs = slice(h * H, (h + 1) * H)
            nc.gpsimd.dma_gather(
                grows[:, gs, :],
                grow.ap(),
                idxw[:, h * 16 : (h + 1) * 16],
                B * NG // 2,
                B * NG // 2,
                D,
                single_packet=False,
            )
            nc.vector.tensor_tensor(
                prod[:, gs, :], grows[:, gs, :], s_pre[:, gs, :], Alu.mult
            )
            nc.sync.dma_start(prod_out.ap()[:, gs, :], prod[:, gs, :])


def _install_ntff_hook():
    """Recreate the antenv.axon_hooks NTFF profile hook this image lacks.

    bass_utils.run_bass_kernel_spmd(trace=True) imports
    antenv.axon_hooks.get_axon_ntff_profile_hook; the axon boot script on this
    image degraded silently because the module is absent. The hook is a thin
    ctypes wrapper over libaxon_pjrt.so's start/stop profile entry points.
    """
    import types, ctypes, contextlib

    if "antenv.axon_hooks" in sys.modules:
        return
    so_path = "/opt/axon/libaxon_pjrt.so"
    try:
        lib = ctypes.CDLL(so_path)
    except OSError:
        return
    if not hasattr(lib, "axon_start_nrt_profile"):
        return
    lib.axon_start_nrt_profile.argtypes = [
        ctypes.POINTER(ctypes.c_int64),
        ctypes.c_size_t,
    ]
    lib.axon_start_nrt_profile.restype = ctypes.c_int64
    lib.axon_stop_nrt_profile.argtypes = [ctypes.c_char_p]
    lib.axon_stop_nrt_profile.restype = ctypes.c_int64

    @contextlib.contextmanager
    def _hook(output_dir, device_ids):
        import jax

        jax.devices()
        if device_ids:
            ids = (ctypes.c_int64 * len(device_ids))(*device_ids)
            rc = lib.axon_start_nrt_profile(ids, len(device_ids))
        else:
            rc = lib.axon_start_nrt_profile(None, 0)
        if rc != 0:
            raise RuntimeError(f"axon_start_nrt_profile rc={rc}")
        try:
            yield
        finally:
            n = lib.axon_stop_nrt_profile(str(output_dir).encode())
            print(f"profile: {n} file(s) written to {output_dir}", file=sys.stderr)

    mod = types.ModuleType("antenv.axon_hooks")
    _state = {"hook": _hook}
    mod.get_axon_ntff_profile_hook = lambda: _state["hook"]
    mod.set_axon_ntff_profile_hook = lambda h: _state.__setitem__("hook", h)
    sys.modules["antenv.axon_hooks"] = mod
    import antenv

    antenv.axon_hooks = mod


_PROGRAM = None


def _get_program():
    global _PROGRAM
    if _PROGRAM is None:
        _PROGRAM = build_program()
    return _PROGRAM


def _prep_inputs(s_f, t_f, gallery):
    import ml_dtypes

    s_f = np.ascontiguousarray(np.asarray(s_f, dtype=np.float32))
    t_f = np.ascontiguousarray(np.asarray(t_f, dtype=np.float32))
    gallery = np.ascontiguousarray(np.asarray(gallery, dtype=np.float32))

    tn = t_f / np.maximum(np.linalg.norm(t_f, axis=1, keepdims=True), 1e-12)
    gn = gallery / np.maximum(np.linalg.norm(gallery, axis=1, keepdims=True), 1e-12)
    tnt = np.ascontiguousarray(tn.T).astype(ml_dtypes.float8_e4m3fn)
    sf16 = s_f.astype(ml_dtypes.bfloat16)

    # s rows pre-permuted to the gather's output layout:
    # spre[p, g, :] = s[8*(p%16) + 2g + p//64, :]
    pp = np.arange(128)
    gg = np.arange(NG)
    bmap = 8 * (pp[:, None] % 16) + 2 * gg[None, :] + pp[:, None] // 64
    spre = np.ascontiguousarray(sf16[bmap])

    p = np.arange(B)[:, None]
    i = np.arange(B)[None, :]
    diag = ((i % 16) == (p % 16)).astype(np.float32)
    in_maps = []
    for c in range(NCORES):
        sl = slice(c * NL, (c + 1) * NL)
        # [D, NL] -> [NT, 128, KC*512]: tile t's DMA reads 4KB contiguous rows
        gsh = gn[sl].T.reshape(KC, 128, NT, 512)
        gsh = np.ascontiguousarray(gsh.transpose(2, 1, 0, 3)).reshape(NT, 128, KC * 512)
        in_maps.append(
            {
                "gnt": gsh.astype(ml_dtypes.float8_e4m3fn),
                "grow": np.ascontiguousarray(gallery[sl]).astype(ml_dtypes.bfloat16),
                "tnt": tnt,
                "spre": spre,
                "diag": diag,
            }
        )
    return in_maps


def kernel(s_f, t_f, gallery, _trace=False):
    if _trace:
        _install_ntff_hook()
    nc = _get_program()
    in_maps = _prep_inputs(s_f, t_f, gallery)
    res = bass_utils.run_bass_kernel_spmd(
        nc, in_maps, core_ids=list(range(NCORES)), trace=_trace
    )

    # host merge: global top-5 per row over the 8 cores' local top-8 lists
    vals = np.stack(
        [res.results[c]["vals"].reshape(B, NC8) for c in range(NCORES)]
    )  # [8, B, 8]
    prods = np.stack(
        [
            np.asarray(res.results[c]["prod"]).astype(np.float32)
            for c in range(NCORES)
        ]
    )  # [8, 128, NG, D]

    v = vals.transpose(1, 0, 2).reshape(B, NCORES * NC8)  # [B, 64]
    top5 = np.argpartition(-v, 5, axis=1)[:, :5]          # [B, 5] flat (core, slot)

    bb = np.arange(B)
    core = top5 // NC8                                    # [B, 5]
    slot = top5 % NC8
    # candidate (b, s<NG) was gathered to prod[64*((b%8)%2) + 16*s + b//8,
    # (b%8)//2, :] on its core
    sl = np.minimum(slot, NG - 1)
    p = 64 * ((bb[:, None] % 8) % 2) + 16 * sl + bb[:, None] // 8
    g = (bb[:, None] % 8) // 2
    drows = prods[core, p, g]                             # [B, 5, D]
    d = np.abs(drows)

    # rare fallback: a winner that ranked >NG-th inside its own core (one core
    # held >=5 of that row's global top-5) was not gathered on device --
    # compute its product row from the staged inputs directly
    rare_b, rare_k = np.nonzero(slot >= NG)
    if rare_b.size:
        idxs = np.stack(
            [res.results[c]["idx"].reshape(B, NC8) for c in range(NCORES)]
        )  # [8, B, 8] local gallery indices as f32
        for b, k in zip(rare_b, rare_k):
            c, s = core[b, k], slot[b, k]
            j = int(idxs[c, b, s]) + c * NL
            d[b, k] = np.abs(gallery[j] * s_f[b])

    m = D // 2
    low_idx = np.argpartition(d, m, axis=-1)[..., :m]     # [B, 5, m]
    member = np.zeros((B, 5, D), dtype=bool)
    member[bb[:, None, None], np.arange(5)[None, :, None], low_idx] = True
    zero_out = member.all(axis=(0, 1))
    mask = np.where(zero_out, 0.0, 1.0).astype(np.float32)

    if _trace:
        kernel.last_exec_time_ns = res.exec_time_ns
        kernel.last_results = res
    return mask
